# revision 23
# baseline (speedup 1.0000x reference)
"""Trainium2 Bass kernel for nn_DataNet (phase-retrieval DataNet step).

Self-contained: hardcodes B=4, L=64, n=1024, patch 256, 8 cores.

Math (validated vs reference in numpy):
  z = img_re + i*img_im ;  z_f = fft2(z*chk) = B1024 z B1024^T   (chk folded via diag(s))
  per patch k at (r,c):  P = z_f[r:r+256, c:c+256] * CTF
    Bz = chk256*ifft2(P) = A256 (P*CTF) A256^T
    V  = Bz * sqrt(Y_k) / |Bz|
    G  = (fft2(V*chk256)) * CTF = (B256 V B256^T) * CTF
  Sfull = sum_k scatter(-G*CTF_scaled) + z_f * W_scaled     (scale = -c0/L folded)
  u' = chk*ifft2(Sfull) = A1024 Sfull A1024^T               (= -c0*u)
  out = (1-c1)*z + c1*Img_a*z/(|z|+eps) + u'
  im_rc = out ; im_ra = |out|

Everything on device lives TRANSPOSED (fused matmul stages compute (M X)^T via
lhsT=data rhs=M^T, so two stages give M X M^T with zero explicit transposes; the
1024-point transforms use split-radix 2x512 with twiddles/checkerboard/scales
folded into the four 512x512 stage matrices and a DVE butterfly).

Sharding: core c -> batch c//2, mask half c%2 (32 patches). Host sums the pair.
"""
import os
import numpy as np

import concourse.bass as bass
import concourse.tile as tile
from concourse import mybir, bacc
from concourse.bass_utils import run_bass_kernel_spmd

N = 1024
H = 512
PN = 256
B = 4
L = 64
LP = 32  # patches per core
EPS = 1e-6

f32 = mybir.dt.float32
f32r = mybir.dt.float32r
i32 = mybir.dt.int32
AF = mybir.ActivationFunctionType
OP = mybir.AluOpType

_PROGRAM_CACHE = {}


class _PhaseDone(Exception):
    pass


def _combos(M):
    """rhs matrices for fused stage out=(M X)^T: [Mr^T, Mi^T, -Mi^T] stacked."""
    Mr = np.ascontiguousarray(M.real.T).astype(np.float32)
    Mi = np.ascontiguousarray(M.imag.T).astype(np.float32)
    return np.stack([Mr, Mi, -Mi])


def _recip(nc, pool, out_t, in_ap, w, tagp, ttag=None):
    """out = 1/in via fast approx + one Newton step (DVE)."""
    r0 = pool.tile([128, w], f32, tag=tagp + "r0")
    nc.vector.reciprocal_approx_fast(r0[:], in_ap)
    t = pool.tile([128, w], f32, tag=(ttag or (tagp + "t")))
    nc.vector.tensor_tensor(out=t[:], in0=in_ap, in1=r0[:], op=OP.mult)
    nc.vector.tensor_scalar(out=t[:], in0=t[:], scalar1=-1.0, scalar2=2.0,
                            op0=OP.mult, op1=OP.add)
    nc.vector.tensor_tensor(out=out_t, in0=r0[:], in1=t[:], op=OP.mult)


def _build_program():
    nc = bacc.Bacc("TRN2", target_bir_lowering=False, debug=False)
    _build_program_inner(nc)
    nc.compile()
    return nc


def _build_program_inner(nc):

    # ---------------- I/O ----------------
    img_re = nc.dram_tensor("img_re", [N, N], f32r, kind="ExternalInput").ap()
    img_im = nc.dram_tensor("img_im", [N, N], f32r, kind="ExternalInput").ap()
    img_a = nc.dram_tensor("img_a", [N, N], f32, kind="ExternalInput").ap()
    yt = nc.dram_tensor("yt", [LP, 2, 128, PN], f32, kind="ExternalInput").ap()
    gidx = nc.dram_tensor("gidx", [LP, 2, 128, 1], i32, kind="ExternalInput").ap()
    wts = nc.dram_tensor("wts", [N, N], f32, kind="ExternalInput").ap()
    ctf_t = nc.dram_tensor("ctf_t", [2, 128, PN], f32, kind="ExternalInput").ap()
    sctf_t = nc.dram_tensor("sctf_t", [2, 128, PN], f32, kind="ExternalInput").ap()
    m256 = nc.dram_tensor("m256", [2, 3, PN, PN], f32r, kind="ExternalInput").ap()
    mfwd = nc.dram_tensor("mfwd", [2, 3, H, H], f32r, kind="ExternalInput").ap()
    minv = nc.dram_tensor("minv", [2, 3, H, H], f32r, kind="ExternalInput").ap()
    csc = nc.dram_tensor("csc", [128, 8, 2], f32, kind="ExternalInput").ap()

    out_re = nc.dram_tensor("out_re", [N, N], f32, kind="ExternalOutput").ap()
    out_im = nc.dram_tensor("out_im", [N, N], f32, kind="ExternalOutput").ap()

    # internal DRAM scratch
    zf_re = nc.dram_tensor("zf_re", [N * N], f32).ap()
    zf_im = nc.dram_tensor("zf_im", [N * N], f32).ap()
    z1_re = nc.dram_tensor("z1_re", [N, N], f32r).ap()
    z1_im = nc.dram_tensor("z1_im", [N, N], f32r).ap()
    acc_re = nc.dram_tensor("acc_re", [N * N], f32r).ap()
    acc_im = nc.dram_tensor("acc_im", [N * N], f32r).ap()

    zf2_re = zf_re.rearrange("(a b) -> a b", b=N)
    zf2_im = zf_im.rearrange("(a b) -> a b", b=N)
    acc2_re = acc_re.rearrange("(a b) -> a b", b=N)
    acc2_im = acc_im.rearrange("(a b) -> a b", b=N)

    with tile.TileContext(nc) as tc:
        with tc.tile_pool(name="consts", bufs=1) as cpool, \
             tc.tile_pool(name="big", bufs=1) as big, \
             tc.tile_pool(name="lhsp", bufs=1) as lhsp, \
             tc.tile_pool(name="work", bufs=1) as work, \
             tc.tile_pool(name="px", bufs=3) as px, \
             tc.tile_pool(name="psum", bufs=2, space="PSUM") as psum, \
             tc.tile_pool(name="ppsum", bufs=2, space="PSUM") as ppsum:
            # ---- load constants ----
            m256_t = cpool.tile([128, 2, 3, 2, PN], f32r)
            nc.sync.dma_start(
                m256_t[:],
                m256.rearrange("m c (k p) n -> p m c k n", p=128))
            ctf_tt = cpool.tile([128, 2, PN], f32)
            nc.sync.dma_start(ctf_tt[:], ctf_t.rearrange("k p n -> p k n"))
            sctf_tt = cpool.tile([128, 2, PN], f32)
            nc.sync.dma_start(sctf_tt[:], sctf_t.rearrange("k p n -> p k n"))
            csc_t = cpool.tile([128, 8, 2], f32)
            nc.sync.dma_start(csc_t[:], csc[:])

            def axis_pass(mats_t, src2_re, src2_im, consume):
                """Per (m, plane): E/O psums then consume(m, pl, e, o)."""
                sre = src2_re.rearrange("(k p two) n -> p k two n", p=128, two=2)
                sim = src2_im.rearrange("(k p two) n -> p k two n", p=128, two=2)
                for m in range(8):
                    lhs = []
                    for v in range(2):
                        lr = lhsp.tile([128, 4, 128], f32r, tag="lhs_r%d" % v)
                        li = lhsp.tile([128, 4, 128], f32r, tag="lhs_i%d" % v)
                        nc.sync.dma_start(lr[:], sre[:, :, v, m * 128:(m + 1) * 128])
                        nc.sync.dma_start(li[:], sim[:, :, v, m * 128:(m + 1) * 128])
                        lhs.append((lr, li))
                    for pl in range(2):
                        ps = {}
                        for v in range(2):
                            lr, li = lhs[v]
                            pt = psum.tile([128, H], f32, tag="fg%d" % v)
                            for kt in range(4):
                                if pl == 0:
                                    nc.tensor.matmul(pt[:], lhsT=lr[:, kt, :], rhs=mats_t[:, v, 0, kt, :],
                                                     start=(kt == 0), stop=False)
                                    nc.tensor.matmul(pt[:], lhsT=li[:, kt, :], rhs=mats_t[:, v, 2, kt, :],
                                                     start=False, stop=(kt == 3))
                                else:
                                    nc.tensor.matmul(pt[:], lhsT=lr[:, kt, :], rhs=mats_t[:, v, 1, kt, :],
                                                     start=(kt == 0), stop=False)
                                    nc.tensor.matmul(pt[:], lhsT=li[:, kt, :], rhs=mats_t[:, v, 0, kt, :],
                                                     start=False, stop=(kt == 3))
                            ps[v] = pt
                        consume(m, pl, ps[0], ps[1])

            def butterfly_plane(e, o, dst, pl):
                """dst[:, 0:512] = E+O ; dst[:, 512:] = E-O (one plane).
                O evacuated into dst's upper half (saves a scratch tile)."""
                nc.scalar.activation(dst[:, H:N], o[:], AF.Copy)
                nc.vector.tensor_tensor(out=dst[:, 0:H], in0=e[:], in1=dst[:, H:N], op=OP.add)
                nc.vector.tensor_tensor(out=dst[:, H:N], in0=e[:], in1=dst[:, H:N], op=OP.subtract)

            # ================= Phase F: forward FFT =================
            mfwd_t = big.tile([128, 2, 3, 4, H], f32r, tag="bigmats")
            nc.sync.dma_start(
                mfwd_t[:], mfwd.rearrange("m c (k p) n -> p m c k n", p=128))

            def f_ax1(m, pl, e, o):
                z = work.tile([128, N], f32r, tag="z1o_%d" % pl)
                butterfly_plane(e, o, z, pl)
                dst = z1_re if pl == 0 else z1_im
                nc.sync.dma_start(dst[m * 128:(m + 1) * 128, :], z[:])

            axis_pass(mfwd_t, img_re, img_im, f_ax1)

            whold = {}

            def f_ax2(m, pl, e, o):
                z = work.tile([128, N], f32, tag="zfo_%d" % pl)
                butterfly_plane(e, o, z, pl)
                dstz = zf2_re if pl == 0 else zf2_im
                nc.sync.dma_start(dstz[m * 128:(m + 1) * 128, :], z[:])
                if pl == 0:
                    wt = work.tile([128, N], f32, tag="wt")
                    nc.sync.dma_start(wt[:], wts[m * 128:(m + 1) * 128, :])
                    whold[m] = wt
                else:
                    wt = whold.pop(m)
                wp = work.tile([128, N], f32r, tag="wp_%d" % pl)
                nc.vector.tensor_tensor(out=wp[:], in0=z[:], in1=wt[:], op=OP.mult)
                dsta = acc2_re if pl == 0 else acc2_im
                nc.sync.dma_start(dsta[m * 128:(m + 1) * 128, :], wp[:])

            axis_pass(mfwd_t, z1_re, z1_im, f_ax2)

            PHASES = int(os.environ.get("K_PHASES", "3"))

            def dump(src_re, src_im):  # debug only
                for m in range(8):
                    t_r = work.tile([128, N], f32, tag="dump_r")
                    t_i = work.tile([128, N], f32, tag="dump_i")
                    nc.gpsimd.dma_start(t_r[:], src_re[m * 128:(m + 1) * 128, :])
                    nc.gpsimd.dma_start(t_i[:], src_im[m * 128:(m + 1) * 128, :])
                    nc.sync.dma_start(out_re[m * 128:(m + 1) * 128, :], t_r[:])
                    nc.sync.dma_start(out_im[m * 128:(m + 1) * 128, :], t_i[:])

            if PHASES == 1:
                dump(zf2_re, zf2_im)
                return

            # ================= Phase P: patches =================
            def patch_stage(xr, xi, mat_idx, tagp):
                """fused 256-stage: out psums [2 tiles [128,2,PN]] = (M X)^T planes."""
                pr = ppsum.tile([128, 2, PN], f32, tag=tagp + "_r")
                pi = ppsum.tile([128, 2, PN], f32, tag=tagp + "_i")
                for ms in range(2):
                    for kt in range(2):
                        xr_s = xr[:, kt, ms * 128:(ms + 1) * 128]
                        xi_s = xi[:, kt, ms * 128:(ms + 1) * 128]
                        nc.tensor.matmul(pr[:, ms, :], lhsT=xr_s, rhs=m256_t[:, mat_idx, 0, kt, :],
                                         start=(kt == 0), stop=False)
                        nc.tensor.matmul(pr[:, ms, :], lhsT=xi_s, rhs=m256_t[:, mat_idx, 2, kt, :],
                                         start=False, stop=(kt == 1))
                        nc.tensor.matmul(pi[:, ms, :], lhsT=xr_s, rhs=m256_t[:, mat_idx, 1, kt, :],
                                         start=(kt == 0), stop=False)
                        nc.tensor.matmul(pi[:, ms, :], lhsT=xi_s, rhs=m256_t[:, mat_idx, 0, kt, :],
                                         start=False, stop=(kt == 1))
                return pr, pi

            for k in range(LP):
                gi = px.tile([128, 2, 1], i32, tag="gi")
                nc.sync.dma_start(gi[:], gidx[k].rearrange("k p one -> p k one"))
                xg_r = px.tile([128, 2, PN], f32, tag="xg_r")
                xg_i = px.tile([128, 2, PN], f32, tag="xg_i")
                for rb in range(2):
                    nc.gpsimd.indirect_dma_start(
                        out=xg_r[:, rb, :], out_offset=None, in_=zf_re[:, None],
                        in_offset=bass.IndirectOffsetOnAxis(ap=gi[:, rb, :], axis=0))
                    nc.gpsimd.indirect_dma_start(
                        out=xg_i[:, rb, :], out_offset=None, in_=zf_im[:, None],
                        in_offset=bass.IndirectOffsetOnAxis(ap=gi[:, rb, :], axis=0))
                # X0 = P^T * CTF^T  (TT rounds into f32r)
                xr = px.tile([128, 2, PN], f32r, tag="xr")
                xi = px.tile([128, 2, PN], f32r, tag="xi")
                nc.vector.tensor_tensor(out=xr[:], in0=xg_r[:], in1=ctf_tt[:], op=OP.mult)
                nc.vector.tensor_tensor(out=xi[:], in0=xg_i[:], in1=ctf_tt[:], op=OP.mult)
                # IFFT stage 1 (A256)
                s1r, s1i = patch_stage(xr, xi, 0, "ss")
                z1r = px.tile([128, 2, PN], f32r, tag="z1r")
                z1i = px.tile([128, 2, PN], f32r, tag="z1i")
                nc.scalar.activation(z1r[:], s1r[:], AF.Copy)
                nc.scalar.activation(z1i[:], s1i[:], AF.Copy)
                # IFFT stage 2 -> Bz in psum
                bzr, bzi = patch_stage(z1r, z1i, 0, "ss")
                # middle: g = sqrt(Y / m2)
                sq_r = px.tile([128, 2, PN], f32, tag="xg_r")
                sq_i = px.tile([128, 2, PN], f32, tag="xg_i")
                nc.scalar.activation(sq_r[:], bzr[:], AF.Square)
                nc.scalar.activation(sq_i[:], bzi[:], AF.Square)
                m2 = px.tile([128, 2, PN], f32, tag="m2")
                nc.vector.tensor_tensor(out=m2.rearrange("p a b -> p (a b)"),
                                        in0=sq_r.rearrange("p a b -> p (a b)"),
                                        in1=sq_i.rearrange("p a b -> p (a b)"), op=OP.add)
                rec = px.tile([128, 2, PN], f32, tag="rec")
                _recip(nc, px, rec.rearrange("p a b -> p (a b)"),
                       m2.rearrange("p a b -> p (a b)"), 2 * PN, "rcp", ttag="q")
                yk = px.tile([128, 2, PN], f32, tag="yk")
                nc.sync.dma_start(yk[:], yt[k].rearrange("k p n -> p k n"))
                q = px.tile([128, 2, PN], f32, tag="m2")
                nc.vector.tensor_tensor(out=q.rearrange("p a b -> p (a b)"),
                                        in0=yk.rearrange("p a b -> p (a b)"),
                                        in1=rec.rearrange("p a b -> p (a b)"), op=OP.mult)
                g = px.tile([128, 2, PN], f32, tag="rec")
                nc.scalar.activation(g.rearrange("p a b -> p (a b)"),
                                     q.rearrange("p a b -> p (a b)"), AF.Sqrt)
                vr = px.tile([128, 2, PN], f32r, tag="xr")
                vi = px.tile([128, 2, PN], f32r, tag="xi")
                for ms in range(2):
                    nc.vector.tensor_tensor(out=vr[:, ms, :], in0=bzr[:, ms, :], in1=g[:, ms, :], op=OP.mult)
                    nc.vector.tensor_tensor(out=vi[:, ms, :], in0=bzi[:, ms, :], in1=g[:, ms, :], op=OP.mult)
                # FFT stage 1+2 (B256)
                t1r, t1i = patch_stage(vr, vi, 1, "ss")
                w1r = px.tile([128, 2, PN], f32r, tag="z1r")
                w1i = px.tile([128, 2, PN], f32r, tag="z1i")
                nc.scalar.activation(w1r[:], t1r[:], AF.Copy)
                nc.scalar.activation(w1i[:], t1i[:], AF.Copy)
                gr_p, gi_p = patch_stage(w1r, w1i, 1, "ss")
                gcr = px.tile([128, 2, PN], f32r, tag="yk")
                gci = px.tile([128, 2, PN], f32r, tag="q")
                for ms in range(2):
                    nc.vector.tensor_tensor(out=gcr[:, ms, :], in0=gr_p[:, ms, :], in1=sctf_tt[:, ms, :], op=OP.mult)
                    nc.vector.tensor_tensor(out=gci[:, ms, :], in0=gi_p[:, ms, :], in1=sctf_tt[:, ms, :], op=OP.mult)
                for rb in range(2):
                    nc.gpsimd.indirect_dma_start(
                        out=acc_re[:, None],
                        out_offset=bass.IndirectOffsetOnAxis(ap=gi[:, rb, :], axis=0),
                        in_=gcr[:, rb, :], in_offset=None, compute_op=OP.add)
                    nc.gpsimd.indirect_dma_start(
                        out=acc_im[:, None],
                        out_offset=bass.IndirectOffsetOnAxis(ap=gi[:, rb, :], axis=0),
                        in_=gci[:, rb, :], in_offset=None, compute_op=OP.add)

            if PHASES == 2:
                dump(acc2_re, acc2_im)
                return

            # ================= Phase I: inverse + base =================
            minv_t = big.tile([128, 2, 3, 4, H], f32r, tag="bigmats")
            nc.sync.dma_start(
                minv_t[:], minv.rearrange("m c (k p) n -> p m c k n", p=128))

            def i_ax1(m, pl, e, o):
                z = work.tile([128, N], f32r, tag="z1o_%d" % pl)
                butterfly_plane(e, o, z, pl)
                dst = z1_re if pl == 0 else z1_im
                nc.sync.dma_start(dst[m * 128:(m + 1) * 128, :], z[:])

            axis_pass(minv_t, acc2_re, acc2_im, i_ax1)

            uhold = {}

            def i_ax2(m, pl, e, o):
                u = work.tile([128, N], f32, tag="u_%d" % pl)
                butterfly_plane(e, o, u, pl)
                if pl == 0:
                    uhold[m] = u
                    return
                ur, ui = uhold.pop(m), u
                if os.environ.get("K_NOBASE"):
                    nc.sync.dma_start(out_re[m * 128:(m + 1) * 128, :], ur[:])
                    nc.sync.dma_start(out_im[m * 128:(m + 1) * 128, :], ui[:])
                    return
                zr = work.tile([128, N], f32r, tag="zfo_0")
                zi = work.tile([128, N], f32r, tag="zfo_1")
                ia = work.tile([128, N], f32, tag="wt")
                nc.sync.dma_start(zr[:], img_re[m * 128:(m + 1) * 128, :])
                nc.sync.dma_start(zi[:], img_im[m * 128:(m + 1) * 128, :])
                nc.sync.dma_start(ia[:], img_a[m * 128:(m + 1) * 128, :])
                s_r = work.tile([128, N], f32, tag="wp_0")
                s_i = work.tile([128, N], f32, tag="wp_1")
                nc.scalar.activation(s_r[:], zr[:], AF.Square)
                nc.scalar.activation(s_i[:], zi[:], AF.Square)
                nc.vector.tensor_tensor(out=s_r[:], in0=s_r[:], in1=s_i[:], op=OP.add)
                nc.scalar.activation(s_r[:], s_r[:], AF.Sqrt)
                nc.vector.tensor_scalar(out=s_r[:], in0=s_r[:], scalar1=EPS, scalar2=None, op0=OP.add)
                _recip(nc, work, s_i[:], s_r[:], N, "brc")
                g2 = s_r
                nc.vector.tensor_tensor(out=g2[:], in0=ia[:], in1=s_i[:], op=OP.mult)
                nc.vector.tensor_tensor(out=g2[:], in0=g2[:],
                                        in1=csc_t[:, m, 0:1].to_broadcast([128, N]),
                                        op=OP.mult)
                nc.vector.tensor_tensor(out=g2[:], in0=g2[:],
                                        in1=csc_t[:, m, 1:2].to_broadcast([128, N]),
                                        op=OP.add)
                nc.vector.tensor_tensor(out=ia[:], in0=zr[:], in1=g2[:], op=OP.mult)
                nc.vector.tensor_tensor(out=ur[:], in0=ia[:], in1=ur[:], op=OP.add)
                nc.vector.tensor_tensor(out=ia[:], in0=zi[:], in1=g2[:], op=OP.mult)
                nc.vector.tensor_tensor(out=ui[:], in0=ia[:], in1=ui[:], op=OP.add)
                nc.sync.dma_start(out_re[m * 128:(m + 1) * 128, :], ur[:])
                nc.sync.dma_start(out_im[m * 128:(m + 1) * 128, :], ui[:])

            axis_pass(minv_t, z1_re, z1_im, i_ax2)

    nc.compile()
    return nc


def _host_prep(Img_a, Img_c_real, Img_c_imag, Y, Masks, CTF, lamb, eta1):
    c0 = 10.0 * float(np.asarray(eta1).reshape(-1)[0])
    c1 = 100.0 * float(np.asarray(eta1).reshape(-1)[0]) * float(np.asarray(lamb).reshape(-1)[0])
    alpha = -c0 / L  # folded into scatter mask and W

    s256 = (-1.0) ** np.arange(PN)
    F256 = np.exp(-2j * np.pi * np.outer(np.arange(PN), np.arange(PN)) / PN)
    A256 = s256[:, None] * np.conj(F256) / PN
    B256 = F256 * s256[None, :]
    m256 = np.stack([_combos(A256), _combos(B256)]).astype(np.float32)  # [2,3,PN,PN]

    s512 = (-1.0) ** np.arange(H)
    j5 = np.arange(H)
    F512 = np.exp(-2j * np.pi * np.outer(j5, j5) / H)
    W1024 = np.exp(-2j * np.pi * j5 / N)
    Wc1024 = np.exp(+2j * np.pi * j5 / N)
    E_f = F512
    O_f = -W1024[:, None] * F512
    mfwd = np.stack([_combos(E_f), _combos(O_f)]).astype(np.float32)
    Fc512 = np.conj(F512)
    E_i = s512[:, None] * Fc512 / N
    O_i = (s512 * 1.0)[:, None] * (Wc1024[:, None] * Fc512) / N
    minv = np.stack([_combos(E_i), _combos(O_i)]).astype(np.float32)

    ctfT = np.ascontiguousarray(CTF.T).astype(np.float32)
    ctf_in = ctfT.reshape(2, 128, PN)
    sctf_in = (alpha * ctfT).reshape(2, 128, PN)  # scatter mask: alpha*(-CTF) ... see sign note

    # sign bookkeeping: acc gets sum(-G*CTF)*(-c0/L)?? We scatter Gc = G * sctf.
    # Want Sfull' = alpha * ( -sum G*CTF + zf*W ), so sctf = -alpha*CTF, W' = alpha*W.
    sctf_in = (-alpha * ctfT).reshape(2, 128, PN)

    CTF2T = np.ascontiguousarray((CTF * CTF).T).astype(np.float32)

    in_maps = []
    for c in range(8):
        b, h = c // 2, c % 2
        ks = np.arange(h * LP, (h + 1) * LP)
        imgT_re = np.ascontiguousarray(Img_c_real[b, 0].T).astype(np.float32)
        imgT_im = np.ascontiguousarray(Img_c_imag[b, 0].T).astype(np.float32)
        imgaT = np.ascontiguousarray(Img_a[b, 0].T).astype(np.float32)
        YTc = np.ascontiguousarray(np.transpose(Y[b, ks], (0, 2, 1))).astype(np.float32)
        yt_in = YTc.reshape(LP, 2, 128, PN)
        gidx_np = np.zeros((LP, 2, 128, 1), np.int32)
        WT = np.zeros((N, N), np.float32)
        for j, k in enumerate(ks):
            r = int(Masks[k, 0]) - 1
            cc = int(Masks[k, 1]) - 1
            rows = cc + np.arange(PN)
            gidx_np[j, :, :, 0] = (rows * N + r).reshape(2, 128)
            WT[cc:cc + PN, r:r + PN] += CTF2T
        WT *= alpha
        cscv = np.zeros((128, 8, 2), np.float32)
        cscv[:, 4 * h:4 * h + 4, 0] = c1
        cscv[:, 4 * h:4 * h + 4, 1] = 1.0 - c1
        in_maps.append(dict(
            img_re=imgT_re, img_im=imgT_im, img_a=imgaT, yt=yt_in,
            gidx=gidx_np, wts=WT, ctf_t=ctf_in, sctf_t=sctf_in,
            m256=m256, mfwd=mfwd, minv=minv, csc=cscv))
    return in_maps


def _run(in_maps, trace=False):
    key = "prog"
    if key not in _PROGRAM_CACHE:
        _PROGRAM_CACHE[key] = _build_program()
    nc = _PROGRAM_CACHE[key]
    if not trace:
        return run_bass_kernel_spmd(nc, in_maps, core_ids=list(range(8)))
    try:
        import axon_profile_shim
        axon_profile_shim.install()
    except Exception:
        pass
    # warm-up run: compiles the NEFF + initializes the PJRT client so the
    # NTFF profile hook can attach on the traced run
    run_bass_kernel_spmd(nc, in_maps, core_ids=list(range(8)))
    try:
        return run_bass_kernel_spmd(nc, in_maps, core_ids=list(range(8)), trace=True)
    except Exception:
        return run_bass_kernel_spmd(nc, in_maps, core_ids=list(range(8)))


def kernel(Img_a, Img_c_real, Img_c_imag, Y, Masks, CTF, lamb, eta1, n1=None, n2=None, T=None):
    Img_a = np.asarray(Img_a, np.float32)
    Img_c_real = np.asarray(Img_c_real, np.float32)
    Img_c_imag = np.asarray(Img_c_imag, np.float32)
    Y = np.asarray(Y, np.float32)
    Masks = np.asarray(Masks)
    CTF = np.asarray(CTF, np.float32)
    in_maps = _host_prep(Img_a, Img_c_real, Img_c_imag, Y, Masks, CTF, lamb, eta1)
    trace = bool(os.environ.get("BASS_KERNEL_TRACE"))
    res = _run(in_maps, trace=trace)
    if trace:
        kernel.last_exec_time_ns = res.exec_time_ns
    im_rc = np.zeros((B, 1, N, N), np.complex64)
    for b in range(B):
        r0 = res.results[2 * b]
        r1 = res.results[2 * b + 1]
        re = (r0["out_re"] + r1["out_re"]).T
        im = (r0["out_im"] + r1["out_im"]).T
        im_rc[b, 0] = re + 1j * im
    im_ra = np.abs(im_rc).astype(np.float32)
    return im_ra, im_rc


# revision 24
# speedup vs baseline: 1.1400x; 1.1400x over previous
"""Trainium2 Bass kernel for nn_DataNet (phase-retrieval DataNet step).

Self-contained: hardcodes B=4, L=64, n=1024, patch 256, 8 cores.

Math (validated vs reference in numpy):
  z = img_re + i*img_im ;  z_f = fft2(z*chk) = B1024 z B1024^T   (chk folded via diag(s))
  per patch k at (r,c):  P = z_f[r:r+256, c:c+256] * CTF
    Bz = chk256*ifft2(P) = A256 (P*CTF) A256^T
    V  = Bz * sqrt(Y_k) / |Bz|
    G  = (fft2(V*chk256)) * CTF = (B256 V B256^T) * CTF
  Sfull = sum_k scatter(-G*CTF_scaled) + z_f * W_scaled     (scale = -c0/L folded)
  u' = chk*ifft2(Sfull) = A1024 Sfull A1024^T               (= -c0*u)
  out = (1-c1)*z + c1*Img_a*z/(|z|+eps) + u'
  im_rc = out ; im_ra = |out|

Everything on device lives TRANSPOSED (fused matmul stages compute (M X)^T via
lhsT=data rhs=M^T, so two stages give M X M^T with zero explicit transposes; the
1024-point transforms use split-radix 2x512 with twiddles/checkerboard/scales
folded into the four 512x512 stage matrices and a DVE butterfly).

Sharding: core c -> batch c//2, mask half c%2 (32 patches). Host sums the pair.
"""
import os
import numpy as np

import concourse.bass as bass
import concourse.tile as tile
from concourse import mybir, bacc
from concourse.bass_utils import run_bass_kernel_spmd

N = 1024
H = 512
PN = 256
B = 4
L = 64
LP = 32  # patches per core
EPS = 1e-6

f32 = mybir.dt.float32
f32r = mybir.dt.float32r
i32 = mybir.dt.int32
AF = mybir.ActivationFunctionType
OP = mybir.AluOpType

_PROGRAM_CACHE = {}


class _PhaseDone(Exception):
    pass


def _combos(M):
    """rhs matrices for fused stage out=(M X)^T: [Mr^T, Mi^T, -Mi^T] stacked."""
    Mr = np.ascontiguousarray(M.real.T).astype(np.float32)
    Mi = np.ascontiguousarray(M.imag.T).astype(np.float32)
    return np.stack([Mr, Mi, -Mi])


def _recip(nc, pool, out_t, in_ap, w, tagp, ttag=None):
    """out = 1/in via fast approx + one Newton step (DVE)."""
    r0 = pool.tile([128, w], f32, tag=tagp + "r0")
    nc.vector.reciprocal_approx_fast(r0[:], in_ap)
    t = pool.tile([128, w], f32, tag=(ttag or (tagp + "t")))
    nc.vector.tensor_tensor(out=t[:], in0=in_ap, in1=r0[:], op=OP.mult)
    nc.vector.tensor_scalar(out=t[:], in0=t[:], scalar1=-1.0, scalar2=2.0,
                            op0=OP.mult, op1=OP.add)
    nc.vector.tensor_tensor(out=out_t, in0=r0[:], in1=t[:], op=OP.mult)


def _build_program():
    nc = bacc.Bacc("TRN2", target_bir_lowering=False, debug=False)
    _build_program_inner(nc)
    nc.compile()
    return nc


def _build_program_inner(nc):

    # ---------------- I/O ----------------
    img_re = nc.dram_tensor("img_re", [N, N], f32r, kind="ExternalInput").ap()
    img_im = nc.dram_tensor("img_im", [N, N], f32r, kind="ExternalInput").ap()
    img_a = nc.dram_tensor("img_a", [N, N], f32, kind="ExternalInput").ap()
    yt = nc.dram_tensor("yt", [LP, 2, 128, PN], f32, kind="ExternalInput").ap()
    gidx = nc.dram_tensor("gidx", [LP, 2, 128, 1], i32, kind="ExternalInput").ap()
    wts = nc.dram_tensor("wts", [N, N], f32, kind="ExternalInput").ap()
    ctf_t = nc.dram_tensor("ctf_t", [2, 128, PN], f32, kind="ExternalInput").ap()
    sctf_t = nc.dram_tensor("sctf_t", [2, 128, PN], f32, kind="ExternalInput").ap()
    m256 = nc.dram_tensor("m256", [2, 3, PN, PN], f32r, kind="ExternalInput").ap()
    mfwd = nc.dram_tensor("mfwd", [2, 3, H, H], f32r, kind="ExternalInput").ap()
    minv = nc.dram_tensor("minv", [2, 3, H, H], f32r, kind="ExternalInput").ap()
    csc = nc.dram_tensor("csc", [128, 8, 2], f32, kind="ExternalInput").ap()

    out_re = nc.dram_tensor("out_re", [N, N], f32, kind="ExternalOutput").ap()
    out_im = nc.dram_tensor("out_im", [N, N], f32, kind="ExternalOutput").ap()

    # internal DRAM scratch
    zf_re = nc.dram_tensor("zf_re", [N * N], f32).ap()
    zf_im = nc.dram_tensor("zf_im", [N * N], f32).ap()
    z1_re = nc.dram_tensor("z1_re", [N, N], f32r).ap()
    z1_im = nc.dram_tensor("z1_im", [N, N], f32r).ap()
    acc_re = nc.dram_tensor("acc_re", [N * N], f32r).ap()
    acc_im = nc.dram_tensor("acc_im", [N * N], f32r).ap()

    zf2_re = zf_re.rearrange("(a b) -> a b", b=N)
    zf2_im = zf_im.rearrange("(a b) -> a b", b=N)
    acc2_re = acc_re.rearrange("(a b) -> a b", b=N)
    acc2_im = acc_im.rearrange("(a b) -> a b", b=N)

    with tile.TileContext(nc) as tc:
        with tc.tile_pool(name="consts", bufs=1) as cpool, \
             tc.tile_pool(name="big", bufs=1) as big, \
             tc.tile_pool(name="lhsp", bufs=1) as lhsp, \
             tc.tile_pool(name="work", bufs=1) as work, \
             tc.tile_pool(name="px", bufs=2) as px, \
             tc.tile_pool(name="psum", bufs=2, space="PSUM") as psum, \
             tc.tile_pool(name="ppsum", bufs=2, space="PSUM") as ppsum:
            # ---- load constants ----
            m256_t = cpool.tile([128, 2, 3, 2, PN], f32r)
            nc.sync.dma_start(
                m256_t[:],
                m256.rearrange("m c (k p) n -> p m c k n", p=128))
            ctf_tt = cpool.tile([128, 2, PN], f32)
            nc.sync.dma_start(ctf_tt[:], ctf_t.rearrange("k p n -> p k n"))
            sctf_tt = cpool.tile([128, 2, PN], f32)
            nc.sync.dma_start(sctf_tt[:], sctf_t.rearrange("k p n -> p k n"))
            csc_t = cpool.tile([128, 8, 2], f32)
            nc.sync.dma_start(csc_t[:], csc[:])

            def axis_pass(mats_t, src2_re, src2_im, consume):
                """Per (m, plane): E/O psums then consume(m, pl, e, o)."""
                sre = src2_re.rearrange("(k p two) n -> p k two n", p=128, two=2)
                sim = src2_im.rearrange("(k p two) n -> p k two n", p=128, two=2)
                for m in range(8):
                    lhs = []
                    for v in range(2):
                        lr = lhsp.tile([128, 4, 128], f32r, tag="lhs_r%d" % v)
                        li = lhsp.tile([128, 4, 128], f32r, tag="lhs_i%d" % v)
                        nc.sync.dma_start(lr[:], sre[:, :, v, m * 128:(m + 1) * 128])
                        nc.sync.dma_start(li[:], sim[:, :, v, m * 128:(m + 1) * 128])
                        lhs.append((lr, li))
                    for pl in range(2):
                        ps = {}
                        for v in range(2):
                            lr, li = lhs[v]
                            pt = psum.tile([128, H], f32, tag="fg%d" % v)
                            for kt in range(4):
                                if pl == 0:
                                    nc.tensor.matmul(pt[:], lhsT=lr[:, kt, :], rhs=mats_t[:, v, 0, kt, :],
                                                     start=(kt == 0), stop=False)
                                    nc.tensor.matmul(pt[:], lhsT=li[:, kt, :], rhs=mats_t[:, v, 2, kt, :],
                                                     start=False, stop=(kt == 3))
                                else:
                                    nc.tensor.matmul(pt[:], lhsT=lr[:, kt, :], rhs=mats_t[:, v, 1, kt, :],
                                                     start=(kt == 0), stop=False)
                                    nc.tensor.matmul(pt[:], lhsT=li[:, kt, :], rhs=mats_t[:, v, 0, kt, :],
                                                     start=False, stop=(kt == 3))
                            ps[v] = pt
                        consume(m, pl, ps[0], ps[1])

            def butterfly_plane(e, o, dst, pl):
                """dst[:, 0:512] = E+O ; dst[:, 512:] = E-O (one plane).
                O evacuated into dst's upper half (saves a scratch tile)."""
                nc.scalar.activation(dst[:, H:N], o[:], AF.Copy)
                nc.vector.tensor_tensor(out=dst[:, 0:H], in0=e[:], in1=dst[:, H:N], op=OP.add)
                nc.vector.tensor_tensor(out=dst[:, H:N], in0=e[:], in1=dst[:, H:N], op=OP.subtract)

            # ================= Phase F: forward FFT =================
            mfwd_t = big.tile([128, 2, 3, 4, H], f32r, tag="bigmats")
            nc.sync.dma_start(
                mfwd_t[:], mfwd.rearrange("m c (k p) n -> p m c k n", p=128))

            def f_ax1(m, pl, e, o):
                z = work.tile([128, N], f32r, tag="z1o_%d" % pl)
                butterfly_plane(e, o, z, pl)
                dst = z1_re if pl == 0 else z1_im
                nc.sync.dma_start(dst[m * 128:(m + 1) * 128, :], z[:])

            axis_pass(mfwd_t, img_re, img_im, f_ax1)

            whold = {}

            def f_ax2(m, pl, e, o):
                z = work.tile([128, N], f32, tag="zfo_%d" % pl)
                butterfly_plane(e, o, z, pl)
                dstz = zf2_re if pl == 0 else zf2_im
                nc.sync.dma_start(dstz[m * 128:(m + 1) * 128, :], z[:])
                if pl == 0:
                    wt = work.tile([128, N], f32, tag="wt")
                    nc.sync.dma_start(wt[:], wts[m * 128:(m + 1) * 128, :])
                    whold[m] = wt
                else:
                    wt = whold.pop(m)
                wp = work.tile([128, N], f32r, tag="wp_%d" % pl)
                nc.vector.tensor_tensor(out=wp[:], in0=z[:], in1=wt[:], op=OP.mult)
                dsta = acc2_re if pl == 0 else acc2_im
                nc.sync.dma_start(dsta[m * 128:(m + 1) * 128, :], wp[:])

            axis_pass(mfwd_t, z1_re, z1_im, f_ax2)

            PHASES = int(os.environ.get("K_PHASES", "3"))

            def dump(src_re, src_im):  # debug only
                for m in range(8):
                    t_r = work.tile([128, N], f32, tag="dump_r")
                    t_i = work.tile([128, N], f32, tag="dump_i")
                    nc.gpsimd.dma_start(t_r[:], src_re[m * 128:(m + 1) * 128, :])
                    nc.gpsimd.dma_start(t_i[:], src_im[m * 128:(m + 1) * 128, :])
                    nc.sync.dma_start(out_re[m * 128:(m + 1) * 128, :], t_r[:])
                    nc.sync.dma_start(out_im[m * 128:(m + 1) * 128, :], t_i[:])

            if PHASES == 1:
                dump(zf2_re, zf2_im)
                return

            # ================= Phase P: patches =================
            def patch_stage(xr, xi, mat_idx, tagp):
                """fused 256-stage: out psums [2 tiles [128,2,PN]] = (M X)^T planes."""
                pr = ppsum.tile([128, 2, PN], f32, tag=tagp + "_r")
                pi = ppsum.tile([128, 2, PN], f32, tag=tagp + "_i")
                for ms in range(2):
                    for kt in range(2):
                        xr_s = xr[:, kt, ms * 128:(ms + 1) * 128]
                        xi_s = xi[:, kt, ms * 128:(ms + 1) * 128]
                        nc.tensor.matmul(pr[:, ms, :], lhsT=xr_s, rhs=m256_t[:, mat_idx, 0, kt, :],
                                         start=(kt == 0), stop=False)
                        nc.tensor.matmul(pr[:, ms, :], lhsT=xi_s, rhs=m256_t[:, mat_idx, 2, kt, :],
                                         start=False, stop=(kt == 1))
                        nc.tensor.matmul(pi[:, ms, :], lhsT=xr_s, rhs=m256_t[:, mat_idx, 1, kt, :],
                                         start=(kt == 0), stop=False)
                        nc.tensor.matmul(pi[:, ms, :], lhsT=xi_s, rhs=m256_t[:, mat_idx, 0, kt, :],
                                         start=False, stop=(kt == 1))
                return pr, pi

            GS = 4  # gather prefetch group
            gtiles = {}

            def issue_gathers(k0):
                for k in range(k0, min(k0 + GS, LP)):
                    sl = k % GS
                    gi = px.tile([128, 2, 1], i32, tag="gi%d" % sl)
                    nc.sync.dma_start(gi[:], gidx[k].rearrange("k p one -> p k one"))
                    xg_r = px.tile([128, 2, PN], f32, tag="xg_r%d" % sl)
                    xg_i = px.tile([128, 2, PN], f32, tag="xg_i%d" % sl)
                    for rb in range(2):
                        nc.gpsimd.indirect_dma_start(
                            out=xg_r[:, rb, :], out_offset=None, in_=zf_re[:, None],
                            in_offset=bass.IndirectOffsetOnAxis(ap=gi[:, rb, :], axis=0))
                        nc.gpsimd.indirect_dma_start(
                            out=xg_i[:, rb, :], out_offset=None, in_=zf_im[:, None],
                            in_offset=bass.IndirectOffsetOnAxis(ap=gi[:, rb, :], axis=0))
                    yk = px.tile([128, 2, PN], f32, tag="yk%d" % sl)
                    nc.sync.dma_start(yk[:], yt[k].rearrange("k p n -> p k n"))
                    gtiles[k] = (gi, xg_r, xg_i, yk)

            issue_gathers(0)
            for k in range(LP):
                if k % GS == 0 and k + GS < LP + 1:
                    if k > 0:
                        issue_gathers(k)
                gi, xg_r, xg_i, yk = gtiles.pop(k)
                # X0 = P^T * CTF^T  (TT rounds into f32r)
                xr = px.tile([128, 2, PN], f32r, tag="xr")
                xi = px.tile([128, 2, PN], f32r, tag="xi")
                nc.vector.tensor_tensor(out=xr[:], in0=xg_r[:], in1=ctf_tt[:], op=OP.mult)
                nc.vector.tensor_tensor(out=xi[:], in0=xg_i[:], in1=ctf_tt[:], op=OP.mult)
                # IFFT stage 1 (A256)
                s1r, s1i = patch_stage(xr, xi, 0, "ss")
                z1r = px.tile([128, 2, PN], f32r, tag="z1r")
                z1i = px.tile([128, 2, PN], f32r, tag="z1i")
                nc.scalar.activation(z1r[:], s1r[:], AF.Copy)
                nc.scalar.activation(z1i[:], s1i[:], AF.Copy)
                # IFFT stage 2 -> Bz in psum
                bzr, bzi = patch_stage(z1r, z1i, 0, "ss")
                # middle: g = sqrt(Y / m2)
                sq_r = px.tile([128, 2, PN], f32, tag="sq_r")
                sq_i = px.tile([128, 2, PN], f32, tag="sq_i")
                nc.scalar.activation(sq_r[:], bzr[:], AF.Square)
                nc.scalar.activation(sq_i[:], bzi[:], AF.Square)
                m2 = px.tile([128, 2, PN], f32, tag="m2")
                nc.vector.tensor_tensor(out=m2.rearrange("p a b -> p (a b)"),
                                        in0=sq_r.rearrange("p a b -> p (a b)"),
                                        in1=sq_i.rearrange("p a b -> p (a b)"), op=OP.add)
                rec = px.tile([128, 2, PN], f32, tag="rec")
                _recip(nc, px, rec.rearrange("p a b -> p (a b)"),
                       m2.rearrange("p a b -> p (a b)"), 2 * PN, "rcp", ttag="q")
                q = px.tile([128, 2, PN], f32, tag="q")
                nc.vector.tensor_tensor(out=q.rearrange("p a b -> p (a b)"),
                                        in0=yk.rearrange("p a b -> p (a b)"),
                                        in1=rec.rearrange("p a b -> p (a b)"), op=OP.mult)
                g = px.tile([128, 2, PN], f32, tag="rec")
                nc.scalar.activation(g.rearrange("p a b -> p (a b)"),
                                     q.rearrange("p a b -> p (a b)"), AF.Sqrt)
                vr = px.tile([128, 2, PN], f32r, tag="xr")
                vi = px.tile([128, 2, PN], f32r, tag="xi")
                for ms in range(2):
                    nc.vector.tensor_tensor(out=vr[:, ms, :], in0=bzr[:, ms, :], in1=g[:, ms, :], op=OP.mult)
                    nc.vector.tensor_tensor(out=vi[:, ms, :], in0=bzi[:, ms, :], in1=g[:, ms, :], op=OP.mult)
                # FFT stage 1+2 (B256)
                t1r, t1i = patch_stage(vr, vi, 1, "ss")
                w1r = px.tile([128, 2, PN], f32r, tag="z1r")
                w1i = px.tile([128, 2, PN], f32r, tag="z1i")
                nc.scalar.activation(w1r[:], t1r[:], AF.Copy)
                nc.scalar.activation(w1i[:], t1i[:], AF.Copy)
                gr_p, gi_p = patch_stage(w1r, w1i, 1, "ss")
                gcr = px.tile([128, 2, PN], f32r, tag="sq_r")
                gci = px.tile([128, 2, PN], f32r, tag="sq_i")
                for ms in range(2):
                    nc.vector.tensor_tensor(out=gcr[:, ms, :], in0=gr_p[:, ms, :], in1=sctf_tt[:, ms, :], op=OP.mult)
                    nc.vector.tensor_tensor(out=gci[:, ms, :], in0=gi_p[:, ms, :], in1=sctf_tt[:, ms, :], op=OP.mult)
                for rb in range(2):
                    nc.gpsimd.indirect_dma_start(
                        out=acc_re[:, None],
                        out_offset=bass.IndirectOffsetOnAxis(ap=gi[:, rb, :], axis=0),
                        in_=gcr[:, rb, :], in_offset=None, compute_op=OP.add)
                    nc.gpsimd.indirect_dma_start(
                        out=acc_im[:, None],
                        out_offset=bass.IndirectOffsetOnAxis(ap=gi[:, rb, :], axis=0),
                        in_=gci[:, rb, :], in_offset=None, compute_op=OP.add)

            # ================= Phase I: inverse + base =================
            minv_t = big.tile([128, 2, 3, 4, H], f32r, tag="bigmats")
            nc.sync.dma_start(
                minv_t[:], minv.rearrange("m c (k p) n -> p m c k n", p=128))

            def i_ax1(m, pl, e, o):
                z = work.tile([128, N], f32r, tag="z1o_%d" % pl)
                butterfly_plane(e, o, z, pl)
                dst = z1_re if pl == 0 else z1_im
                nc.sync.dma_start(dst[m * 128:(m + 1) * 128, :], z[:])

            axis_pass(minv_t, acc2_re, acc2_im, i_ax1)

            uhold = {}

            def i_ax2(m, pl, e, o):
                u = work.tile([128, N], f32, tag="u_%d" % pl)
                butterfly_plane(e, o, u, pl)
                if pl == 0:
                    uhold[m] = u
                    return
                ur, ui = uhold.pop(m), u
                if os.environ.get("K_NOBASE"):
                    nc.sync.dma_start(out_re[m * 128:(m + 1) * 128, :], ur[:])
                    nc.sync.dma_start(out_im[m * 128:(m + 1) * 128, :], ui[:])
                    return
                zr = work.tile([128, N], f32r, tag="zfo_0")
                zi = work.tile([128, N], f32r, tag="zfo_1")
                ia = work.tile([128, N], f32, tag="wt")
                nc.sync.dma_start(zr[:], img_re[m * 128:(m + 1) * 128, :])
                nc.sync.dma_start(zi[:], img_im[m * 128:(m + 1) * 128, :])
                nc.sync.dma_start(ia[:], img_a[m * 128:(m + 1) * 128, :])
                s_r = work.tile([128, N], f32, tag="wp_0")
                s_i = work.tile([128, N], f32, tag="wp_1")
                nc.scalar.activation(s_r[:], zr[:], AF.Square)
                nc.scalar.activation(s_i[:], zi[:], AF.Square)
                nc.vector.tensor_tensor(out=s_r[:], in0=s_r[:], in1=s_i[:], op=OP.add)
                nc.scalar.activation(s_r[:], s_r[:], AF.Sqrt)
                nc.vector.tensor_scalar(out=s_r[:], in0=s_r[:], scalar1=EPS, scalar2=None, op0=OP.add)
                _recip(nc, work, s_i[:], s_r[:], N, "brc")
                g2 = s_r
                nc.vector.tensor_tensor(out=g2[:], in0=ia[:], in1=s_i[:], op=OP.mult)
                nc.vector.tensor_tensor(out=g2[:], in0=g2[:],
                                        in1=csc_t[:, m, 0:1].to_broadcast([128, N]),
                                        op=OP.mult)
                nc.vector.tensor_tensor(out=g2[:], in0=g2[:],
                                        in1=csc_t[:, m, 1:2].to_broadcast([128, N]),
                                        op=OP.add)
                nc.vector.tensor_tensor(out=ia[:], in0=zr[:], in1=g2[:], op=OP.mult)
                nc.vector.tensor_tensor(out=ur[:], in0=ia[:], in1=ur[:], op=OP.add)
                nc.vector.tensor_tensor(out=ia[:], in0=zi[:], in1=g2[:], op=OP.mult)
                nc.vector.tensor_tensor(out=ui[:], in0=ia[:], in1=ui[:], op=OP.add)
                nc.sync.dma_start(out_re[m * 128:(m + 1) * 128, :], ur[:])
                nc.sync.dma_start(out_im[m * 128:(m + 1) * 128, :], ui[:])

            axis_pass(minv_t, z1_re, z1_im, i_ax2)

    nc.compile()
    return nc


def _host_prep(Img_a, Img_c_real, Img_c_imag, Y, Masks, CTF, lamb, eta1):
    c0 = 10.0 * float(np.asarray(eta1).reshape(-1)[0])
    c1 = 100.0 * float(np.asarray(eta1).reshape(-1)[0]) * float(np.asarray(lamb).reshape(-1)[0])
    alpha = -c0 / L  # folded into scatter mask and W

    s256 = (-1.0) ** np.arange(PN)
    F256 = np.exp(-2j * np.pi * np.outer(np.arange(PN), np.arange(PN)) / PN)
    A256 = s256[:, None] * np.conj(F256) / PN
    B256 = F256 * s256[None, :]
    m256 = np.stack([_combos(A256), _combos(B256)]).astype(np.float32)  # [2,3,PN,PN]

    s512 = (-1.0) ** np.arange(H)
    j5 = np.arange(H)
    F512 = np.exp(-2j * np.pi * np.outer(j5, j5) / H)
    W1024 = np.exp(-2j * np.pi * j5 / N)
    Wc1024 = np.exp(+2j * np.pi * j5 / N)
    E_f = F512
    O_f = -W1024[:, None] * F512
    mfwd = np.stack([_combos(E_f), _combos(O_f)]).astype(np.float32)
    Fc512 = np.conj(F512)
    E_i = s512[:, None] * Fc512 / N
    O_i = (s512 * 1.0)[:, None] * (Wc1024[:, None] * Fc512) / N
    minv = np.stack([_combos(E_i), _combos(O_i)]).astype(np.float32)

    ctfT = np.ascontiguousarray(CTF.T).astype(np.float32)
    ctf_in = ctfT.reshape(2, 128, PN)
    sctf_in = (alpha * ctfT).reshape(2, 128, PN)  # scatter mask: alpha*(-CTF) ... see sign note

    # sign bookkeeping: acc gets sum(-G*CTF)*(-c0/L)?? We scatter Gc = G * sctf.
    # Want Sfull' = alpha * ( -sum G*CTF + zf*W ), so sctf = -alpha*CTF, W' = alpha*W.
    sctf_in = (-alpha * ctfT).reshape(2, 128, PN)

    CTF2T = np.ascontiguousarray((CTF * CTF).T).astype(np.float32)

    in_maps = []
    for c in range(8):
        b, h = c // 2, c % 2
        ks = np.arange(h * LP, (h + 1) * LP)
        imgT_re = np.ascontiguousarray(Img_c_real[b, 0].T).astype(np.float32)
        imgT_im = np.ascontiguousarray(Img_c_imag[b, 0].T).astype(np.float32)
        imgaT = np.ascontiguousarray(Img_a[b, 0].T).astype(np.float32)
        YTc = np.ascontiguousarray(np.transpose(Y[b, ks], (0, 2, 1))).astype(np.float32)
        yt_in = YTc.reshape(LP, 2, 128, PN)
        gidx_np = np.zeros((LP, 2, 128, 1), np.int32)
        WT = np.zeros((N, N), np.float32)
        for j, k in enumerate(ks):
            r = int(Masks[k, 0]) - 1
            cc = int(Masks[k, 1]) - 1
            rows = cc + np.arange(PN)
            gidx_np[j, :, :, 0] = (rows * N + r).reshape(2, 128)
            WT[cc:cc + PN, r:r + PN] += CTF2T
        WT *= alpha
        cscv = np.zeros((128, 8, 2), np.float32)
        cscv[:, 4 * h:4 * h + 4, 0] = c1
        cscv[:, 4 * h:4 * h + 4, 1] = 1.0 - c1
        in_maps.append(dict(
            img_re=imgT_re, img_im=imgT_im, img_a=imgaT, yt=yt_in,
            gidx=gidx_np, wts=WT, ctf_t=ctf_in, sctf_t=sctf_in,
            m256=m256, mfwd=mfwd, minv=minv, csc=cscv))
    return in_maps


def _run(in_maps, trace=False):
    key = "prog"
    if key not in _PROGRAM_CACHE:
        _PROGRAM_CACHE[key] = _build_program()
    nc = _PROGRAM_CACHE[key]
    if not trace:
        return run_bass_kernel_spmd(nc, in_maps, core_ids=list(range(8)))
    try:
        import axon_profile_shim
        axon_profile_shim.install()
    except Exception:
        pass
    # warm-up run: compiles the NEFF + initializes the PJRT client so the
    # NTFF profile hook can attach on the traced run
    run_bass_kernel_spmd(nc, in_maps, core_ids=list(range(8)))
    try:
        return run_bass_kernel_spmd(nc, in_maps, core_ids=list(range(8)), trace=True)
    except Exception:
        return run_bass_kernel_spmd(nc, in_maps, core_ids=list(range(8)))


def kernel(Img_a, Img_c_real, Img_c_imag, Y, Masks, CTF, lamb, eta1, n1=None, n2=None, T=None):
    Img_a = np.asarray(Img_a, np.float32)
    Img_c_real = np.asarray(Img_c_real, np.float32)
    Img_c_imag = np.asarray(Img_c_imag, np.float32)
    Y = np.asarray(Y, np.float32)
    Masks = np.asarray(Masks)
    CTF = np.asarray(CTF, np.float32)
    in_maps = _host_prep(Img_a, Img_c_real, Img_c_imag, Y, Masks, CTF, lamb, eta1)
    trace = bool(os.environ.get("BASS_KERNEL_TRACE"))
    res = _run(in_maps, trace=trace)
    if trace:
        kernel.last_exec_time_ns = res.exec_time_ns
    im_rc = np.zeros((B, 1, N, N), np.complex64)
    for b in range(B):
        r0 = res.results[2 * b]
        r1 = res.results[2 * b + 1]
        re = (r0["out_re"] + r1["out_re"]).T
        im = (r0["out_im"] + r1["out_im"]).T
        im_rc[b, 0] = re + 1j * im
    im_ra = np.abs(im_rc).astype(np.float32)
    return im_ra, im_rc


# revision 26
# speedup vs baseline: 1.1538x; 1.0122x over previous
"""Trainium2 Bass kernel for nn_DataNet (phase-retrieval DataNet step).

Self-contained: hardcodes B=4, L=64, n=1024, patch 256, 8 cores.

Math (validated vs reference in numpy):
  z = img_re + i*img_im ;  z_f = fft2(z*chk) = B1024 z B1024^T   (chk folded via diag(s))
  per patch k at (r,c):  P = z_f[r:r+256, c:c+256] * CTF
    Bz = chk256*ifft2(P) = A256 (P*CTF) A256^T
    V  = Bz * sqrt(Y_k) / |Bz|
    G  = (fft2(V*chk256)) * CTF = (B256 V B256^T) * CTF
  Sfull = sum_k scatter(-G*CTF_scaled) + z_f * W_scaled     (scale = -c0/L folded)
  u' = chk*ifft2(Sfull) = A1024 Sfull A1024^T               (= -c0*u)
  out = (1-c1)*z + c1*Img_a*z/(|z|+eps) + u'
  im_rc = out ; im_ra = |out|

Everything on device lives TRANSPOSED (fused matmul stages compute (M X)^T via
lhsT=data rhs=M^T, so two stages give M X M^T with zero explicit transposes; the
1024-point transforms use split-radix 2x512 with twiddles/checkerboard/scales
folded into the four 512x512 stage matrices and a DVE butterfly).

Sharding: core c -> batch c//2, mask half c%2 (32 patches). Host sums the pair.
"""
import os
import numpy as np

import concourse.bass as bass
import concourse.tile as tile
from concourse import mybir, bacc
from concourse.bass_utils import run_bass_kernel_spmd

N = 1024
H = 512
PN = 256
B = 4
L = 64
LP = 32  # patches per core
EPS = 1e-6

f32 = mybir.dt.float32
f32r = mybir.dt.float32r
i32 = mybir.dt.int32
AF = mybir.ActivationFunctionType
OP = mybir.AluOpType

_PROGRAM_CACHE = {}


class _PhaseDone(Exception):
    pass


def _combos(M):
    """rhs matrices for fused stage out=(M X)^T: [Mr^T, Mi^T, -Mi^T] stacked."""
    Mr = np.ascontiguousarray(M.real.T).astype(np.float32)
    Mi = np.ascontiguousarray(M.imag.T).astype(np.float32)
    return np.stack([Mr, Mi, -Mi])


def _recip(nc, pool, out_t, in_ap, w, tagp, ttag=None):
    """out = 1/in via fast approx + one Newton step (DVE)."""
    r0 = pool.tile([128, w], f32, tag=tagp + "r0")
    nc.vector.reciprocal_approx_fast(r0[:], in_ap)
    t = pool.tile([128, w], f32, tag=(ttag or (tagp + "t")))
    nc.vector.tensor_tensor(out=t[:], in0=in_ap, in1=r0[:], op=OP.mult)
    nc.vector.tensor_scalar(out=t[:], in0=t[:], scalar1=-1.0, scalar2=2.0,
                            op0=OP.mult, op1=OP.add)
    nc.vector.tensor_tensor(out=out_t, in0=r0[:], in1=t[:], op=OP.mult)


def _build_program():
    nc = bacc.Bacc("TRN2", target_bir_lowering=False, debug=False)
    _build_program_inner(nc)
    nc.compile()
    return nc


def _build_program_inner(nc):

    # ---------------- I/O ----------------
    img_re = nc.dram_tensor("img_re", [N, N], f32r, kind="ExternalInput").ap()
    img_im = nc.dram_tensor("img_im", [N, N], f32r, kind="ExternalInput").ap()
    img_a = nc.dram_tensor("img_a", [N, N], f32, kind="ExternalInput").ap()
    yt = nc.dram_tensor("yt", [LP, 2, 128, PN], f32, kind="ExternalInput").ap()
    gidx = nc.dram_tensor("gidx", [LP, 2, 128, 1], i32, kind="ExternalInput").ap()
    wts = nc.dram_tensor("wts", [N, N], f32, kind="ExternalInput").ap()
    ctf_t = nc.dram_tensor("ctf_t", [2, 128, PN], f32, kind="ExternalInput").ap()
    sctf_t = nc.dram_tensor("sctf_t", [2, 128, PN], f32, kind="ExternalInput").ap()
    m256 = nc.dram_tensor("m256", [2, 3, PN, PN], f32r, kind="ExternalInput").ap()
    mfwd = nc.dram_tensor("mfwd", [2, 3, H, H], f32r, kind="ExternalInput").ap()
    minv = nc.dram_tensor("minv", [2, 3, H, H], f32r, kind="ExternalInput").ap()
    csc = nc.dram_tensor("csc", [128, 8, 2], f32, kind="ExternalInput").ap()

    out_re = nc.dram_tensor("out_re", [N, N], f32, kind="ExternalOutput").ap()
    out_im = nc.dram_tensor("out_im", [N, N], f32, kind="ExternalOutput").ap()

    # internal DRAM scratch
    zf_re = nc.dram_tensor("zf_re", [N * N], f32).ap()
    zf_im = nc.dram_tensor("zf_im", [N * N], f32).ap()
    z1_re = nc.dram_tensor("z1_re", [N, N], f32r).ap()
    z1_im = nc.dram_tensor("z1_im", [N, N], f32r).ap()
    acc_re = nc.dram_tensor("acc_re", [N * N], f32r).ap()
    acc_im = nc.dram_tensor("acc_im", [N * N], f32r).ap()

    zf2_re = zf_re.rearrange("(a b) -> a b", b=N)
    zf2_im = zf_im.rearrange("(a b) -> a b", b=N)
    acc2_re = acc_re.rearrange("(a b) -> a b", b=N)
    acc2_im = acc_im.rearrange("(a b) -> a b", b=N)

    with tile.TileContext(nc) as tc:
        with tc.tile_pool(name="consts", bufs=1) as cpool, \
             tc.tile_pool(name="big", bufs=1) as big, \
             tc.tile_pool(name="lhsp", bufs=1) as lhsp, \
             tc.tile_pool(name="work", bufs=1) as work, \
             tc.tile_pool(name="px", bufs=2) as px, \
             tc.tile_pool(name="psum", bufs=2, space="PSUM") as psum, \
             tc.tile_pool(name="ppsum", bufs=2, space="PSUM") as ppsum:
            # ---- load constants ----
            m256_t = cpool.tile([128, 2, 3, 2, PN], f32r)
            nc.sync.dma_start(
                m256_t[:],
                m256.rearrange("m c (k p) n -> p m c k n", p=128))
            ctf_tt = cpool.tile([128, 2, PN], f32)
            nc.sync.dma_start(ctf_tt[:], ctf_t.rearrange("k p n -> p k n"))
            sctf_tt = cpool.tile([128, 2, PN], f32)
            nc.sync.dma_start(sctf_tt[:], sctf_t.rearrange("k p n -> p k n"))
            csc_t = cpool.tile([128, 8, 2], f32)
            nc.sync.dma_start(csc_t[:], csc[:])

            def axis_pass(mats_t, src2_re, src2_im, consume):
                """Per (m, plane): E/O psums then consume(m, pl, e, o)."""
                sre = src2_re.rearrange("(k p two) n -> p k two n", p=128, two=2)
                sim = src2_im.rearrange("(k p two) n -> p k two n", p=128, two=2)
                for m in range(8):
                    lhs = []
                    for v in range(2):
                        lr = lhsp.tile([128, 4, 128], f32r, tag="lhs_r%d" % v)
                        li = lhsp.tile([128, 4, 128], f32r, tag="lhs_i%d" % v)
                        nc.sync.dma_start(lr[:], sre[:, :, v, m * 128:(m + 1) * 128])
                        nc.sync.dma_start(li[:], sim[:, :, v, m * 128:(m + 1) * 128])
                        lhs.append((lr, li))
                    for pl in range(2):
                        ps = {}
                        for v in range(2):
                            lr, li = lhs[v]
                            pt = psum.tile([128, H], f32, tag="fg%d" % v)
                            for kt in range(4):
                                if pl == 0:
                                    nc.tensor.matmul(pt[:], lhsT=lr[:, kt, :], rhs=mats_t[:, v, 0, kt, :],
                                                     start=(kt == 0), stop=False)
                                    nc.tensor.matmul(pt[:], lhsT=li[:, kt, :], rhs=mats_t[:, v, 2, kt, :],
                                                     start=False, stop=(kt == 3))
                                else:
                                    nc.tensor.matmul(pt[:], lhsT=lr[:, kt, :], rhs=mats_t[:, v, 1, kt, :],
                                                     start=(kt == 0), stop=False)
                                    nc.tensor.matmul(pt[:], lhsT=li[:, kt, :], rhs=mats_t[:, v, 0, kt, :],
                                                     start=False, stop=(kt == 3))
                            ps[v] = pt
                        consume(m, pl, ps[0], ps[1])

            def butterfly_plane(e, o, dst, pl):
                """dst[:, 0:512] = E+O ; dst[:, 512:] = E-O (one plane).
                O evacuated into dst's upper half (saves a scratch tile)."""
                nc.scalar.activation(dst[:, H:N], o[:], AF.Copy)
                nc.vector.tensor_tensor(out=dst[:, 0:H], in0=e[:], in1=dst[:, H:N], op=OP.add)
                nc.vector.tensor_tensor(out=dst[:, H:N], in0=e[:], in1=dst[:, H:N], op=OP.subtract)

            # ================= Phase F: forward FFT =================
            mfwd_t = big.tile([128, 2, 3, 4, H], f32r, tag="bigmats")
            nc.sync.dma_start(
                mfwd_t[:], mfwd.rearrange("m c (k p) n -> p m c k n", p=128))

            def f_ax1(m, pl, e, o):
                z = work.tile([128, N], f32r, tag="z1o_%d" % pl)
                butterfly_plane(e, o, z, pl)
                dst = z1_re if pl == 0 else z1_im
                nc.sync.dma_start(dst[m * 128:(m + 1) * 128, :], z[:])

            axis_pass(mfwd_t, img_re, img_im, f_ax1)

            whold = {}

            def f_ax2(m, pl, e, o):
                z = work.tile([128, N], f32, tag="zfo_%d" % pl)
                butterfly_plane(e, o, z, pl)
                dstz = zf2_re if pl == 0 else zf2_im
                nc.sync.dma_start(dstz[m * 128:(m + 1) * 128, :], z[:])
                if pl == 0:
                    wt = work.tile([128, N], f32, tag="wt")
                    nc.sync.dma_start(wt[:], wts[m * 128:(m + 1) * 128, :])
                    whold[m] = wt
                else:
                    wt = whold.pop(m)
                wp = work.tile([128, N], f32r, tag="wp_%d" % pl)
                nc.vector.tensor_tensor(out=wp[:], in0=z[:], in1=wt[:], op=OP.mult)
                dsta = acc2_re if pl == 0 else acc2_im
                nc.sync.dma_start(dsta[m * 128:(m + 1) * 128, :], wp[:])

            axis_pass(mfwd_t, z1_re, z1_im, f_ax2)

            PHASES = int(os.environ.get("K_PHASES", "3"))

            def dump(src_re, src_im):  # debug only
                for m in range(8):
                    t_r = work.tile([128, N], f32, tag="dump_r")
                    t_i = work.tile([128, N], f32, tag="dump_i")
                    nc.gpsimd.dma_start(t_r[:], src_re[m * 128:(m + 1) * 128, :])
                    nc.gpsimd.dma_start(t_i[:], src_im[m * 128:(m + 1) * 128, :])
                    nc.sync.dma_start(out_re[m * 128:(m + 1) * 128, :], t_r[:])
                    nc.sync.dma_start(out_im[m * 128:(m + 1) * 128, :], t_i[:])

            if PHASES == 1:
                dump(zf2_re, zf2_im)
                return

            # ================= Phase P: patches =================
            def patch_stage(xr, xi, mat_idx, tagp):
                """fused 256-stage: out psums [2 tiles [128,2,PN]] = (M X)^T planes."""
                pr = ppsum.tile([128, 2, PN], f32, tag=tagp + "_r")
                pi = ppsum.tile([128, 2, PN], f32, tag=tagp + "_i")
                for ms in range(2):
                    for kt in range(2):
                        xr_s = xr[:, kt, ms * 128:(ms + 1) * 128]
                        xi_s = xi[:, kt, ms * 128:(ms + 1) * 128]
                        nc.tensor.matmul(pr[:, ms, :], lhsT=xr_s, rhs=m256_t[:, mat_idx, 0, kt, :],
                                         start=(kt == 0), stop=False)
                        nc.tensor.matmul(pr[:, ms, :], lhsT=xi_s, rhs=m256_t[:, mat_idx, 2, kt, :],
                                         start=False, stop=(kt == 1))
                        nc.tensor.matmul(pi[:, ms, :], lhsT=xr_s, rhs=m256_t[:, mat_idx, 1, kt, :],
                                         start=(kt == 0), stop=False)
                        nc.tensor.matmul(pi[:, ms, :], lhsT=xi_s, rhs=m256_t[:, mat_idx, 0, kt, :],
                                         start=False, stop=(kt == 1))
                return pr, pi

            GS = 4  # gather prefetch group
            gtiles = {}

            def issue_gathers(k0):
                for k in range(k0, min(k0 + GS, LP)):
                    sl = k % GS
                    gi = px.tile([128, 2, 1], i32, tag="gi%d" % sl)
                    nc.sync.dma_start(gi[:], gidx[k].rearrange("k p one -> p k one"))
                    xg_r = px.tile([128, 2, PN], f32, tag="xg_r%d" % sl)
                    xg_i = px.tile([128, 2, PN], f32, tag="xg_i%d" % sl)
                    for rb in range(2):
                        nc.gpsimd.indirect_dma_start(
                            out=xg_r[:, rb, :], out_offset=None, in_=zf_re[:, None],
                            in_offset=bass.IndirectOffsetOnAxis(ap=gi[:, rb, :], axis=0))
                        nc.gpsimd.indirect_dma_start(
                            out=xg_i[:, rb, :], out_offset=None, in_=zf_im[:, None],
                            in_offset=bass.IndirectOffsetOnAxis(ap=gi[:, rb, :], axis=0))
                    yk = px.tile([128, 2, PN], f32, tag="yk%d" % sl)
                    nc.sync.dma_start(yk[:], yt[k].rearrange("k p n -> p k n"))
                    gtiles[k] = (gi, xg_r, xg_i, yk)

            issue_gathers(0)
            for k in range(LP):
                if k % GS == 0 and k + GS < LP + 1:
                    if k > 0:
                        issue_gathers(k)
                gi, xg_r, xg_i, yk = gtiles.pop(k)
                # X0 = P^T * CTF^T  (TT rounds into f32r)
                xr = px.tile([128, 2, PN], f32r, tag="xr")
                xi = px.tile([128, 2, PN], f32r, tag="xi")
                nc.vector.tensor_tensor(out=xr[:], in0=xg_r[:], in1=ctf_tt[:], op=OP.mult)
                nc.vector.tensor_tensor(out=xi[:], in0=xg_i[:], in1=ctf_tt[:], op=OP.mult)
                # IFFT stage 1 (A256)
                s1r, s1i = patch_stage(xr, xi, 0, "ss")
                z1r = px.tile([128, 2, PN], f32r, tag="z1r")
                z1i = px.tile([128, 2, PN], f32r, tag="z1i")
                nc.scalar.activation(z1r[:], s1r[:], AF.Copy)
                nc.scalar.activation(z1i[:], s1i[:], AF.Copy)
                # IFFT stage 2 -> Bz in psum
                bzr, bzi = patch_stage(z1r, z1i, 0, "ss")
                # middle: g = sqrt(Y / m2)
                sq_r = px.tile([128, 2, PN], f32, tag="sq_r")
                sq_i = px.tile([128, 2, PN], f32, tag="sq_i")
                nc.scalar.activation(sq_r[:], bzr[:], AF.Square)
                nc.scalar.activation(sq_i[:], bzi[:], AF.Square)
                m2 = px.tile([128, 2, PN], f32, tag="m2")
                nc.vector.tensor_tensor(out=m2.rearrange("p a b -> p (a b)"),
                                        in0=sq_r.rearrange("p a b -> p (a b)"),
                                        in1=sq_i.rearrange("p a b -> p (a b)"), op=OP.add)
                rec = px.tile([128, 2, PN], f32, tag="rec")
                _recip(nc, px, rec.rearrange("p a b -> p (a b)"),
                       m2.rearrange("p a b -> p (a b)"), 2 * PN, "rcp", ttag="q")
                q = px.tile([128, 2, PN], f32, tag="q")
                nc.vector.tensor_tensor(out=q.rearrange("p a b -> p (a b)"),
                                        in0=yk.rearrange("p a b -> p (a b)"),
                                        in1=rec.rearrange("p a b -> p (a b)"), op=OP.mult)
                g = px.tile([128, 2, PN], f32, tag="rec")
                nc.scalar.activation(g.rearrange("p a b -> p (a b)"),
                                     q.rearrange("p a b -> p (a b)"), AF.Sqrt)
                vr = px.tile([128, 2, PN], f32r, tag="xr")
                vi = px.tile([128, 2, PN], f32r, tag="xi")
                for ms in range(2):
                    nc.vector.tensor_tensor(out=vr[:, ms, :], in0=bzr[:, ms, :], in1=g[:, ms, :], op=OP.mult)
                    nc.vector.tensor_tensor(out=vi[:, ms, :], in0=bzi[:, ms, :], in1=g[:, ms, :], op=OP.mult)
                # FFT stage 1+2 (B256)
                t1r, t1i = patch_stage(vr, vi, 1, "ss")
                w1r = px.tile([128, 2, PN], f32r, tag="z1r")
                w1i = px.tile([128, 2, PN], f32r, tag="z1i")
                nc.scalar.activation(w1r[:], t1r[:], AF.Copy)
                nc.scalar.activation(w1i[:], t1i[:], AF.Copy)
                gr_p, gi_p = patch_stage(w1r, w1i, 1, "ss")
                gcr = px.tile([128, 2, PN], f32r, tag="sq_r")
                gci = px.tile([128, 2, PN], f32r, tag="sq_i")
                for ms in range(2):
                    nc.vector.tensor_tensor(out=gcr[:, ms, :], in0=gr_p[:, ms, :], in1=sctf_tt[:, ms, :], op=OP.mult)
                    nc.vector.tensor_tensor(out=gci[:, ms, :], in0=gi_p[:, ms, :], in1=sctf_tt[:, ms, :], op=OP.mult)
                for rb in range(2):
                    nc.gpsimd.indirect_dma_start(
                        out=acc_re[:, None],
                        out_offset=bass.IndirectOffsetOnAxis(ap=gi[:, rb, :], axis=0),
                        in_=gcr[:, rb, :], in_offset=None, compute_op=OP.add)
                    nc.gpsimd.indirect_dma_start(
                        out=acc_im[:, None],
                        out_offset=bass.IndirectOffsetOnAxis(ap=gi[:, rb, :], axis=0),
                        in_=gci[:, rb, :], in_offset=None, compute_op=OP.add)

            # ================= Phase I: inverse + base =================
            minv_t = big.tile([128, 2, 3, 4, H], f32r, tag="bigmats")
            nc.sync.dma_start(
                minv_t[:], minv.rearrange("m c (k p) n -> p m c k n", p=128))

            def i_ax1(m, pl, e, o):
                z = work.tile([128, N], f32r, tag="z1o_%d" % pl)
                butterfly_plane(e, o, z, pl)
                dst = z1_re if pl == 0 else z1_im
                nc.sync.dma_start(dst[m * 128:(m + 1) * 128, :], z[:])

            axis_pass(minv_t, acc2_re, acc2_im, i_ax1)

            uhold = {}

            def i_ax2(m, pl, e, o):
                u = work.tile([128, N], f32, tag="u_%d" % pl)
                butterfly_plane(e, o, u, pl)
                if pl == 0:
                    uhold[m] = u
                    return
                ur, ui = uhold.pop(m), u
                if os.environ.get("K_NOBASE"):
                    nc.sync.dma_start(out_re[m * 128:(m + 1) * 128, :], ur[:])
                    nc.sync.dma_start(out_im[m * 128:(m + 1) * 128, :], ui[:])
                    return
                zr = work.tile([128, N], f32r, tag="zfo_0")
                zi = work.tile([128, N], f32r, tag="zfo_1")
                ia = work.tile([128, N], f32, tag="wt")
                nc.sync.dma_start(zr[:], img_re[m * 128:(m + 1) * 128, :])
                nc.sync.dma_start(zi[:], img_im[m * 128:(m + 1) * 128, :])
                nc.sync.dma_start(ia[:], img_a[m * 128:(m + 1) * 128, :])
                s_r = work.tile([128, N], f32, tag="wp_0")
                s_i = work.tile([128, N], f32, tag="wp_1")
                nc.scalar.activation(s_r[:], zr[:], AF.Square)
                nc.scalar.activation(s_i[:], zi[:], AF.Square)
                nc.vector.tensor_tensor(out=s_r[:], in0=s_r[:], in1=s_i[:], op=OP.add)
                nc.scalar.activation(s_r[:], s_r[:], AF.Sqrt)
                nc.vector.tensor_scalar(out=s_r[:], in0=s_r[:], scalar1=EPS, scalar2=None, op0=OP.add)
                _recip(nc, work, s_i[:], s_r[:], N, "brc")
                g2 = s_r
                nc.vector.tensor_tensor(out=g2[:], in0=ia[:], in1=s_i[:], op=OP.mult)
                nc.vector.tensor_tensor(out=g2[:], in0=g2[:],
                                        in1=csc_t[:, m, 0:1].to_broadcast([128, N]),
                                        op=OP.mult)
                nc.vector.tensor_tensor(out=g2[:], in0=g2[:],
                                        in1=csc_t[:, m, 1:2].to_broadcast([128, N]),
                                        op=OP.add)
                nc.vector.tensor_tensor(out=ia[:], in0=zr[:], in1=g2[:], op=OP.mult)
                nc.vector.tensor_tensor(out=ur[:], in0=ia[:], in1=ur[:], op=OP.add)
                nc.vector.tensor_tensor(out=ia[:], in0=zi[:], in1=g2[:], op=OP.mult)
                nc.vector.tensor_tensor(out=ui[:], in0=ia[:], in1=ui[:], op=OP.add)
                nc.sync.dma_start(out_re[m * 128:(m + 1) * 128, :], ur[:])
                nc.sync.dma_start(out_im[m * 128:(m + 1) * 128, :], ui[:])

            axis_pass(minv_t, z1_re, z1_im, i_ax2)

    nc.compile()
    return nc


def _host_prep(Img_a, Img_c_real, Img_c_imag, Y, Masks, CTF, lamb, eta1):
    c0 = 10.0 * float(np.asarray(eta1).reshape(-1)[0])
    c1 = 100.0 * float(np.asarray(eta1).reshape(-1)[0]) * float(np.asarray(lamb).reshape(-1)[0])
    alpha = -c0 / L  # folded into scatter mask and W

    s256 = (-1.0) ** np.arange(PN)
    F256 = np.exp(-2j * np.pi * np.outer(np.arange(PN), np.arange(PN)) / PN)
    A256 = s256[:, None] * np.conj(F256) / PN
    B256 = F256 * s256[None, :]
    m256 = np.stack([_combos(A256), _combos(B256)]).astype(np.float32)  # [2,3,PN,PN]

    s512 = (-1.0) ** np.arange(H)
    j5 = np.arange(H)
    F512 = np.exp(-2j * np.pi * np.outer(j5, j5) / H)
    W1024 = np.exp(-2j * np.pi * j5 / N)
    Wc1024 = np.exp(+2j * np.pi * j5 / N)
    E_f = F512
    O_f = -W1024[:, None] * F512
    mfwd = np.stack([_combos(E_f), _combos(O_f)]).astype(np.float32)
    Fc512 = np.conj(F512)
    E_i = s512[:, None] * Fc512 / N
    O_i = (s512 * 1.0)[:, None] * (Wc1024[:, None] * Fc512) / N
    minv = np.stack([_combos(E_i), _combos(O_i)]).astype(np.float32)

    ctfT = np.ascontiguousarray(CTF.T).astype(np.float32)
    ctf_in = ctfT.reshape(2, 128, PN)
    sctf_in = (alpha * ctfT).reshape(2, 128, PN)  # scatter mask: alpha*(-CTF) ... see sign note

    # sign bookkeeping: acc gets sum(-G*CTF)*(-c0/L)?? We scatter Gc = G * sctf.
    # Want Sfull' = alpha * ( -sum G*CTF + zf*W ), so sctf = -alpha*CTF, W' = alpha*W.
    sctf_in = (-alpha * ctfT).reshape(2, 128, PN)

    CTF2T = np.ascontiguousarray((CTF * CTF).T).astype(np.float32)

    in_maps = []
    for c in range(8):
        b, h = c // 2, c % 2
        ks = np.arange(h * LP, (h + 1) * LP)
        imgT_re = np.ascontiguousarray(Img_c_real[b, 0].T).astype(np.float32)
        imgT_im = np.ascontiguousarray(Img_c_imag[b, 0].T).astype(np.float32)
        imgaT = np.ascontiguousarray(Img_a[b, 0].T).astype(np.float32)
        YTc = np.ascontiguousarray(np.transpose(Y[b, ks], (0, 2, 1))).astype(np.float32)
        yt_in = YTc.reshape(LP, 2, 128, PN)
        gidx_np = np.zeros((LP, 2, 128, 1), np.int32)
        WT = np.zeros((N, N), np.float32)
        for j, k in enumerate(ks):
            r = int(Masks[k, 0]) - 1
            cc = int(Masks[k, 1]) - 1
            rows = cc + np.arange(PN)
            gidx_np[j, :, :, 0] = (rows * N + r).reshape(2, 128)
            WT[cc:cc + PN, r:r + PN] += CTF2T
        WT *= alpha
        cscv = np.zeros((128, 8, 2), np.float32)
        cscv[:, 4 * h:4 * h + 4, 0] = c1
        cscv[:, 4 * h:4 * h + 4, 1] = 1.0 - c1
        in_maps.append(dict(
            img_re=imgT_re, img_im=imgT_im, img_a=imgaT, yt=yt_in,
            gidx=gidx_np, wts=WT, ctf_t=ctf_in, sctf_t=sctf_in,
            m256=m256, mfwd=mfwd, minv=minv, csc=cscv))
    return in_maps


def _run(in_maps, trace=False):
    key = "prog"
    if key not in _PROGRAM_CACHE:
        _PROGRAM_CACHE[key] = _build_program()
    nc = _PROGRAM_CACHE[key]
    if not trace:
        return run_bass_kernel_spmd(nc, in_maps, core_ids=list(range(8)))
    try:
        import axon_profile_shim
        axon_profile_shim.install()
    except Exception:
        pass
    # warm-up run: compiles the NEFF + initializes the PJRT client so the
    # NTFF profile hook can attach on the traced run
    run_bass_kernel_spmd(nc, in_maps, core_ids=list(range(8)))
    try:
        return run_bass_kernel_spmd(nc, in_maps, core_ids=list(range(8)), trace=True)
    except Exception:
        return run_bass_kernel_spmd(nc, in_maps, core_ids=list(range(8)))


def kernel(Img_a, Img_c_real, Img_c_imag, Y, Masks, CTF, lamb, eta1, n1=None, n2=None, T=None):
    Img_a = np.asarray(Img_a, np.float32)
    Img_c_real = np.asarray(Img_c_real, np.float32)
    Img_c_imag = np.asarray(Img_c_imag, np.float32)
    Y = np.asarray(Y, np.float32)
    Masks = np.asarray(Masks)
    CTF = np.asarray(CTF, np.float32)
    in_maps = _host_prep(Img_a, Img_c_real, Img_c_imag, Y, Masks, CTF, lamb, eta1)
    trace = bool(os.environ.get("BASS_KERNEL_TRACE"))
    res = _run(in_maps, trace=trace)
    if trace:
        kernel.last_exec_time_ns = res.exec_time_ns
    im_rc = np.zeros((B, 1, N, N), np.complex64)
    for b in range(B):
        r0 = res.results[2 * b]
        r1 = res.results[2 * b + 1]
        re = (r0["out_re"] + r1["out_re"]).T
        im = (r0["out_im"] + r1["out_im"]).T
        im_rc[b, 0] = re + 1j * im
    im_ra = np.abs(im_rc).astype(np.float32)
    return im_ra, im_rc
